# revision 1
# baseline (speedup 1.0000x reference)
"""AutoAdaptiveFocalLossV2 on 8 Trainium2 NeuronCores.

Math per row r of input [N, C]:
    s      = sum_c exp(x[r, c])                  (no max-subtraction: x ~ randn, bounded)
    xt     = x[r, target[r]]
    logpt  = xt - log(s)
    pt     = exp(logpt)
    bin    = searchsorted(edges, pt, 'right') = sum_i [pt >= e_i]
    gamma  = g[bin] = g[0] + sum_i (g[i+1]-g[i]) * [pt >= e_i]
    loss_r = -(1 - pt + 1e-20)^gamma * logpt = -exp(gamma*log1m) * logpt
Output = sum_r loss_r.

Sharding: pure data-parallel, 16384 rows per core.

Layout trick: sum(exp(x)) is invariant to within-row permutation, so the host
swaps x[r, 0] <-> x[r, target[r]] while sharding (index-driven data movement,
like the resharding itself). On device the "gather" is then a strided read of
column 0, which removes the 160 us/core DVE masked-gather pass entirely.

Per 2 MB chunk (4 row-blocks of 128 rows; host pre-interleaved so each
partition's chunk block is one contiguous 16 KB DMA descriptor):
  - HWDGE streams x (~65.5 MB/core total; ~160-170 us at achieved HBM rate).
  - ACT computes exp for every block; for 1 in 4 blocks it also row-sums via
    accum_out. The other 3 blocks are row-summed on DVE (tensor_reduce of the
    exp tile). The 3:1 split balances ACT (~151 us) and DVE (~137 us) near
    the DMA floor.
  - DVE also copies the 4 swapped-target elements (column 0 of each block).
The binning epilogue runs in two halves; the first half overlaps the second
half of the stream. Per-core output is a [128] partial sum; host sums 1024
values in f64. Measured: ~191 us/core best (HBM roofline ~183 us), rel err
~6e-8 vs the fp32 reference.

The single-sync-wait workarounds (sink/sinka/joiner ops, post-pass wait
rewrites) exist because this walrus build refuses any instruction carrying
more than one semaphore wait.
"""

import os
import numpy as np

N = 131072
C = 1000
NUM_BINS = 15
P = 128
NCORES = 8
RPC = N // NCORES          # 16384 rows per core
COLS = RPC // P            # 128 staging columns (one per 128-row block)
J = 4                      # row-blocks per DMA chunk (2 MB per dma_start)
CHUNKS = COLS // J         # 32 chunks per core
ACC_EVERY = 4              # 1 in 4 blocks row-summed on ACT (accum); rest on DVE

LAST_RESULT = None         # BassKernelResults of the most recent run (for test.py)


def build_program(bin_edges, bin_gammas, hw_fixups=True):
    import concourse.bass as bass
    import concourse.mybir as mybir
    import concourse.tile as tile

    f32 = mybir.dt.float32
    Alu = mybir.AluOpType
    Act = mybir.ActivationFunctionType

    edges = [float(v) for v in np.asarray(bin_edges, np.float64)]
    gammas = [float(v) for v in np.asarray(bin_gammas, np.float64)]
    assert len(edges) == NUM_BINS - 1 and len(gammas) == NUM_BINS

    nc = bass.Bass()
    x_d = nc.dram_tensor("xin", [RPC, C], f32, kind="ExternalInput")
    out_d = nc.dram_tensor("out", [P, 1], f32, kind="ExternalOutput")

    # Host pre-interleaves the shard so partition p's J row-blocks of chunk k
    # are contiguous in DRAM (one 16 KB descriptor per partition per chunk):
    # DRAM order [k][p][j][c]  <->  logical row k*J*128 + j*128 + p
    x_re = x_d[:, :].rearrange("(k p e) c -> k p (e c)", p=P, e=J)

    with tile.TileContext(nc) as tc:
        with (
            tc.tile_pool(name="xpool", bufs=8) as xpool,
            tc.tile_pool(name="scratch", bufs=2) as scratch,
            tc.tile_pool(name="epool", bufs=4) as epool,
            tc.tile_pool(name="consts", bufs=1) as consts,
            tc.tile_pool(name="stage", bufs=1) as stage,
        ):
            ones = consts.tile([P, 1], f32, tag="ones")
            nc.vector.memset(ones[:], 1.0)

            # separate per-engine row-sum staging (an ACT accum write and a
            # DVE reduce write into the same SBUF tile crashed the exec unit);
            # merged by strided copies in the epilogue, so no init needed.
            s_acc = stage.tile([P, COLS], f32, tag="s_acc")
            s_dve = stage.tile([P, COLS], f32, tag="s_dve")
            xt_all = stage.tile([P, COLS], f32, tag="xt_all")

            # epilogue machinery (defined up front; half 0 is traced in the
            # middle of the streaming loop so its ops overlap chunks 16-31)
            def dve_absorb(src_ap):
                t = scratch.tile([P, 1], f32, tag="eabs")
                nc.vector.tensor_tensor(
                    out=t[:], in0=src_ap, in1=ones[:], op=Alu.add
                )

            HALF = COLS // 2
            parts = stage.tile([P, 2], f32, tag="parts")

            def epilogue_half(h):
                lo, hi = h * HALF, (h + 1) * HALF
                # merge the row-sum stagings: every 4th column came from ACT,
                # the rest from DVE (strided copies; each target column is
                # written exactly once so no init needed)
                s_all = stage.tile([P, HALF], f32, tag=f"s_all{h}")
                sa4 = s_acc[:, lo:hi].rearrange("p (a b) -> p a b", b=ACC_EVERY)
                sd4 = s_dve[:, lo:hi].rearrange("p (a b) -> p a b", b=ACC_EVERY)
                sm4 = s_all[:].rearrange("p (a b) -> p a b", b=ACC_EVERY)
                dve_absorb(s_acc[:, lo : lo + 1])
                nc.vector.tensor_copy(sm4[:, :, 0:1], sa4[:, :, 0:1])
                nc.vector.tensor_copy(sm4[:, :, 1:ACC_EVERY], sd4[:, :, 1:ACC_EVERY])

                ln_s = stage.tile([P, HALF], f32, tag=f"ln_s{h}")
                nc.scalar.activation(out=ln_s[:], in_=s_all[:], func=Act.Ln)
                logpt = stage.tile([P, HALF], f32, tag=f"logpt{h}")
                dve_absorb(ln_s[:, 0:1])
                nc.vector.tensor_tensor(
                    out=logpt[:], in0=xt_all[:, lo:hi], in1=ln_s[:],
                    op=Alu.subtract,
                )
                pt = stage.tile([P, HALF], f32, tag=f"pt{h}")
                nc.scalar.activation(out=pt[:], in_=logpt[:], func=Act.Exp)
                om = stage.tile([P, HALF], f32, tag=f"om{h}")  # 1 - pt
                nc.scalar.activation(
                    out=om[:], in_=pt[:], func=Act.Copy, scale=-1.0, bias=1.0
                )
                ln1m = stage.tile([P, HALF], f32, tag=f"ln1m{h}")
                nc.scalar.activation(out=ln1m[:], in_=om[:], func=Act.Ln)

                gamma = stage.tile([P, HALF], f32, tag=f"gamma{h}")
                nc.vector.memset(gamma[:], gammas[0])
                mtmp = stage.tile([P, HALF], f32, tag=f"mtmp{h}")
                for i in range(NUM_BINS - 1):
                    dg = gammas[i + 1] - gammas[i]
                    nc.vector.tensor_scalar(
                        out=mtmp[:], in0=pt[:], scalar1=edges[i], scalar2=dg,
                        op0=Alu.is_ge, op1=Alu.mult,
                    )
                    nc.vector.tensor_tensor(
                        out=gamma[:], in0=gamma[:], in1=mtmp[:], op=Alu.add
                    )

                prod = stage.tile([P, HALF], f32, tag=f"prod{h}")
                dve_absorb(ln1m[:, 0:1])
                nc.vector.tensor_tensor(
                    out=prod[:], in0=gamma[:], in1=ln1m[:], op=Alu.mult
                )
                focal = stage.tile([P, HALF], f32, tag=f"focal{h}")
                nc.scalar.activation(out=focal[:], in_=prod[:], func=Act.Exp)
                contrib = stage.tile([P, HALF], f32, tag=f"contrib{h}")
                dve_absorb(focal[:, 0:1])
                nc.vector.tensor_tensor(
                    out=contrib[:], in0=focal[:], in1=logpt[:], op=Alu.mult
                )
                nc.vector.tensor_reduce(
                    out=parts[:, h : h + 1], in_=contrib[:],
                    axis=mybir.AxisListType.X, op=Alu.add, negate=True,
                )

            # main streaming loop. Chunk 0 is issued as J separate block DMAs
            # with per-block absorbers so the first exp starts after ~0.5 MB.
            for k in range(CHUNKS):
                x_t = xpool.tile([P, J, C], f32, tag="x")
                if k == 0:
                    for j in range(J):
                        nc.sync.dma_start(
                            out=x_t[:, j, :],
                            in_=x_re[0][:, j * C : (j + 1) * C],
                        )
                        sink = scratch.tile([P, 1], f32, tag="sink")
                        nc.vector.tensor_tensor(
                            out=sink[:], in0=x_t[:, j, 0:1], in1=ones[:],
                            op=Alu.add,
                        )
                else:
                    nc.sync.dma_start(out=x_t[:], in_=x_re[k])
                    # sink/sinka absorb the chunk's DMA wait for each engine
                    # so later ops carry at most one sync wait each.
                    sink = scratch.tile([P, 1], f32, tag="sink")
                    nc.vector.tensor_tensor(
                        out=sink[:], in0=x_t[:, 0, 0:1], in1=ones[:], op=Alu.add
                    )
                # swapped targets: column 0 of every block, one strided copy
                nc.vector.tensor_copy(
                    xt_all[:, k * J : (k + 1) * J], x_t[:, :, 0]
                )
                for j in range(J):
                    col = k * J + j
                    e_t = epool.tile([P, C], f32, tag="exp_out")
                    if j % ACC_EVERY == 0:
                        nc.scalar.activation(
                            out=e_t[:], in_=x_t[:, j, :], func=Act.Exp,
                            accum_out=s_acc[:, col : col + 1],
                        )
                    else:
                        nc.scalar.activation(
                            out=e_t[:], in_=x_t[:, j, :], func=Act.Exp
                        )
                        nc.vector.tensor_reduce(
                            out=s_dve[:, col : col + 1], in_=e_t[:],
                            axis=mybir.AxisListType.X, op=Alu.add,
                        )
                # cross-engine joiner: last DVE accessor of this x slot. Its
                # in0 column was produced by this chunk's last DVE reduce,
                # which itself waited on the chunk's last ACT exp, so the
                # joiner's completion implies every reader of the slot is done
                # and the slot-recycling DMA can carry a single DVE wait.
                last_col = k * J + J - 1
                joiner = scratch.tile([P, 1], f32, tag="joiner")
                nc.vector.tensor_tensor(
                    out=joiner[:],
                    in0=s_dve[:, last_col : last_col + 1],
                    in1=x_t[:, 0, 0:1],
                    op=Alu.add,
                )
                if k == CHUNKS // 2 - 1:
                    # first epilogue half overlaps the remaining streaming
                    epilogue_half(0)

            epilogue_half(1)
            part = stage.tile([P, 1], f32, tag="part")
            nc.vector.tensor_tensor(
                out=part[:], in0=parts[:, 0:1], in1=parts[:, 1:2], op=Alu.add
            )
            nc.sync.dma_start(out=out_d[:, :], in_=part[:])

    if hw_fixups:
        # (skipped for CoreSim: its race detector can't execute hand-edited
        #  sync rewrites; they only change synchronization, not data flow)
        apply_hw_fixups(nc, mybir)
    return nc


def apply_hw_fixups(nc, mybir):
    # Tile piggybacks an own-engine semaphore wait onto any instruction that
    # carries a cross-engine wait. Engines execute and complete their queue
    # in order (the DVE even drains its pipe between ops), so a wait on the
    # instruction's own engine semaphore is always redundant — strip it.
    own_prefix = {
        "EngineType.DVE": "DVE",
        "EngineType.Activation": "Activation",
        "EngineType.Pool": "Pool",
        "EngineType.PE": "PE",
        "EngineType.SP": "SP",
    }
    for blk in nc.m.functions[0].blocks:
        for ins in blk.instructions:
            si = getattr(ins, "sync_info", None)
            if si is None or type(ins).__name__ == "InstDMACopy":
                continue
            if len(si.on_wait) <= 1:
                continue
            pref = own_prefix.get(str(getattr(ins, "engine", "")), None)
            if pref is None:
                continue
            keep = [w for w in si.on_wait if not w.ant_name.startswith(pref + "_")]
            if len(keep) < len(si.on_wait):
                ins.sync_info = type(si)(on_wait=keep, on_update=list(si.on_update))

    # walrus' DMA instruction encoding holds a single sync wait. Tile puts
    # up to three on the steady-state streaming DMAs: the recycled slot's ACT
    # readers, its DVE readers, and its previous DMA writer (WAW). The DVE
    # wait alone is sufficient: the last DVE accessor is the per-chunk joiner
    # above, whose completion transitively implies the ACT readers and (via
    # the in-order DVE queue and the sink's DMA wait) the previous writer.
    for blk in nc.m.functions[0].blocks:
        for ins in blk.instructions:
            si = getattr(ins, "sync_info", None)
            if si is None or type(ins).__name__ != "InstDMACopy":
                continue
            if len(si.on_wait) <= 1:
                continue
            keep = [w for w in si.on_wait if w.ant_name.startswith("DVE")]
            assert len(keep) == 1, (ins.name, [w.ant_name for w in si.on_wait])
            ins.sync_info = type(si)(on_wait=keep, on_update=list(si.on_update))

    # The kernel-tail drain aggregates one wait per semaphore in a single
    # instruction; split it into a chain of single-wait drains on the same
    # engine (sequential execution preserves the barrier semantics).
    for blk in nc.m.functions[0].blocks:
        il = blk.instructions
        i = 0
        while i < len(il):
            ins = il[i]
            si = getattr(ins, "sync_info", None)
            if (
                si is not None
                and type(ins).__name__ == "InstDrain"
                and len(si.on_wait) > 1
            ):
                SyncInfo = type(si)
                waits = list(si.on_wait)
                for k, w in enumerate(waits[:-1]):
                    d = mybir.InstDrain(
                        name=f"{ins.name}-w{k}", ins=[], outs=[],
                        bass_is_fusable=False,
                    )
                    d.engine = ins.engine
                    d.sync_info = SyncInfo(on_wait=[w], on_update=[])
                    il.insert(i, d)
                    i += 1
                ins.sync_info = SyncInfo(
                    on_wait=[waits[-1]], on_update=list(si.on_update)
                )
            i += 1


def make_in_maps(input, target):
    x = np.asarray(input, dtype=np.float32).copy()
    t = np.asarray(target).astype(np.int64)
    # swap x[r, 0] <-> x[r, target[r]]: after this, column 0 holds the target
    # logit and the row's multiset (hence sum(exp)) is unchanged.
    rows = np.arange(N)
    v0 = x[rows, 0].copy()
    vt = x[rows, t].copy()
    x[rows, 0] = vt
    x[rows, t] = v0
    in_maps = []
    for c in range(NCORES):
        xs = x[c * RPC : (c + 1) * RPC]
        # interleave to DRAM order [k][p][j][c] so each partition's chunk
        # block is contiguous (row k*J*128 + j*128 + p -> [k][p][j])
        xi = np.ascontiguousarray(
            xs.reshape(CHUNKS, J, P, C).transpose(0, 2, 1, 3).reshape(RPC, C)
        )
        in_maps.append({"xin": xi})
    return in_maps


def kernel(input, target, bin_edges, bin_gammas):
    global LAST_RESULT
    from concourse.bass_utils import run_bass_kernel_spmd

    nc = build_program(bin_edges, bin_gammas)
    in_maps = make_in_maps(input, target)
    trace = bool(os.environ.get("BASS_TRACE"))
    res = run_bass_kernel_spmd(nc, in_maps, list(range(NCORES)), trace=trace)
    LAST_RESULT = res
    total = np.float64(0.0)
    for r in res.results:
        total += r["out"].astype(np.float64).sum()
    return np.float32(total)



# revision 3
# speedup vs baseline: 1.4550x; 1.4550x over previous
"""AutoAdaptiveFocalLossV2 on 8 Trainium2 NeuronCores.

Math per row r of input [N, C]:
    s      = sum_c exp(x[r, c])                  (no max-subtraction: x ~ randn, bounded)
    logpt  = x[r, target[r]] - log(s)
    pt     = exp(logpt)
    bin    = searchsorted(edges, pt, 'right') = sum_i [pt >= e_i]
    gamma  = g[bin] = g[0] + sum_i (g[i+1]-g[i]) * [pt >= e_i]
    loss_r = -(1 - pt + 1e-20)^gamma * logpt = -exp(gamma*log1m) * logpt
Output = sum_r loss_r.

Sharding: pure data-parallel, 16384 rows per core.

The rel-err budget (2e-2) is large, so the kernel streams the logits in
bf16 (halving HBM traffic vs f32: ~32.8 MB/core) and splits sum(exp(x))
row-block by row-block across two engines:
  - ACT blocks: activation(Exp, accum_out=...) does exp+row-sum in one
    instruction (~1.3 us/block incl. the accumulator read).
  - DVE blocks: a Schraudolph exp2 — tensor_scalar computes
    i16 = int16(x*(2^7*log2e) + (127*2^7 + adj)) whose bit pattern IS
    bf16(exp(x)) to within +-3% sawtooth error (adj centers the row-sum
    mean; residual total-loss error ~4e-5).  The int16 output engages the
    DVE 4x perf mode (0.25 elem/cycle); a bitcast-to-bf16 tensor_reduce
    then row-sums the block.
The target logit x[r, t[r]] is gathered on the host (index-driven data
movement, like the resharding) and DMA'd as a tiny fp32 side tensor, so
logpt keeps full precision.

Per 1 MB chunk (4 row-blocks of 128 rows; host pre-interleaved so each
partition's chunk block is one contiguous 8 KB DMA descriptor). Binning
epilogue in two halves; first half overlaps the second half of the
stream. Per-core output is a [128] partial sum; host sums in f64.

The single-sync-wait workarounds (joiner ops, post-pass wait rewrites)
exist because this walrus build refuses any instruction carrying more
than one semaphore wait.
"""

import os
import numpy as np

N = 131072
C = 1000
NUM_BINS = 15
P = 128
NCORES = 8
RPC = N // NCORES          # 16384 rows per core
COLS = RPC // P            # 128 row-blocks (one staging column each)
J = 4                      # row-blocks per DMA chunk (1 MB per dma_start)
CHUNKS = COLS // J         # 32 chunks per core

# Schraudolph exp2-in-bf16-bits constants (see docstring).
SCH_C1 = 1.4426950408889634 * 128.0            # log2(e) * 2^7
SCH_C2 = 127.0 * 128.0 - 7.4                   # bias + sawtooth centering

LAST_RESULT = None         # BassKernelResults of the most recent run (for test.py)


def block_on_dve(col):
    """Engine assignment per row-block: chunks are [D,A,D,A], except
    chunks with k%8 in {1,5,6} are [D,A,A,A] -> 52 DVE / 76 ACT blocks."""
    j, k = col % J, col // J
    if j == 0:
        return True
    return j == 2 and (k % 8) not in (1, 5, 6)


def build_program(bin_edges, bin_gammas, hw_fixups=True):
    import concourse.bass as bass
    import concourse.mybir as mybir
    import concourse.tile as tile

    f32 = mybir.dt.float32
    bf16 = mybir.dt.bfloat16
    i16 = mybir.dt.int16
    Alu = mybir.AluOpType
    Act = mybir.ActivationFunctionType

    edges = [float(v) for v in np.asarray(bin_edges, np.float64)]
    gammas = [float(v) for v in np.asarray(bin_gammas, np.float64)]
    assert len(edges) == NUM_BINS - 1 and len(gammas) == NUM_BINS

    nc = bass.Bass()
    x_d = nc.dram_tensor("xin", [RPC, C], bf16, kind="ExternalInput")
    xt_d = nc.dram_tensor("xt", [P, COLS], f32, kind="ExternalInput")
    out_d = nc.dram_tensor("out", [P, 1], f32, kind="ExternalOutput")

    # Host pre-interleaves the shard so partition p's J row-blocks of chunk k
    # are contiguous in DRAM (one 8 KB descriptor per partition per chunk):
    # DRAM order [k][p][j][c]  <->  logical row k*J*128 + j*128 + p
    x_re = x_d[:, :].rearrange("(k p e) c -> k p (e c)", p=P, e=J)

    with tile.TileContext(nc) as tc:
        with (
            tc.tile_pool(name="xpool", bufs=8) as xpool,
            tc.tile_pool(name="scratch", bufs=2) as scratch,
            tc.tile_pool(name="epool", bufs=2) as epool,
            tc.tile_pool(name="ipool", bufs=2) as ipool,
            tc.tile_pool(name="consts", bufs=1) as consts,
            tc.tile_pool(name="stage", bufs=1) as stage,
        ):
            ones = consts.tile([P, 1], f32, tag="ones")
            nc.vector.memset(ones[:], 1.0)

            # separate per-engine row-sum staging (an ACT accum write and a
            # DVE reduce write into the same SBUF tile crashed the exec
            # unit); merged by ACT strided copies in the epilogue.
            s_acc = stage.tile([P, COLS], f32, tag="s_acc")
            s_dve = stage.tile([P, COLS], f32, tag="s_dve")
            xt_all = stage.tile([P, COLS], f32, tag="xt_all")

            # target-logit side channel: one small DMA; the DVE sink absorbs
            # its completion wait so later readers carry no DMA wait.
            nc.sync.dma_start(out=xt_all[:], in_=xt_d[:, :])
            sink0 = scratch.tile([P, 1], f32, tag="sink")
            nc.vector.tensor_tensor(
                out=sink0[:], in0=xt_all[:, 0:1], in1=ones[:], op=Alu.add
            )

            def dve_absorb(src_ap):
                t = scratch.tile([P, 1], f32, tag="eabs")
                nc.vector.tensor_tensor(
                    out=t[:], in0=src_ap, in1=ones[:], op=Alu.add
                )

            HALF = COLS // 2
            parts = stage.tile([P, 2], f32, tag="parts")

            def epilogue_half(h):
                lo, hi = h * HALF, (h + 1) * HALF
                # Merge row-sum stagings on ACT (s_all only ever written by
                # ACT): bulk-copy the ACT staging, then overwrite the DVE
                # columns (j==0 of every chunk; j==2 of chunks k%8 in
                # {0,2,3,4,7}) with strided copies.
                s_all = stage.tile([P, HALF], f32, tag=f"s_all{h}")
                nc.scalar.copy(s_all[:], s_acc[:, lo:hi])
                sd4 = s_dve[:, lo:hi].rearrange("p (a b) -> p a b", b=J)
                sm4 = s_all[:].rearrange("p (a b) -> p a b", b=J)
                nc.scalar.copy(sm4[:, :, 0:1], sd4[:, :, 0:1])
                sd32 = s_dve[:, lo:hi].rearrange("p (a k b) -> p a k b", k=8, b=J)
                sm32 = s_all[:].rearrange("p (a k b) -> p a k b", k=8, b=J)
                for kk in (0, 2, 3, 4, 7):
                    nc.scalar.copy(sm32[:, :, kk, 2:3], sd32[:, :, kk, 2:3])

                ln_s = stage.tile([P, HALF], f32, tag=f"ln_s{h}")
                nc.scalar.activation(out=ln_s[:], in_=s_all[:], func=Act.Ln)
                logpt = stage.tile([P, HALF], f32, tag=f"logpt{h}")
                dve_absorb(ln_s[:, 0:1])
                nc.vector.tensor_tensor(
                    out=logpt[:], in0=xt_all[:, lo:hi], in1=ln_s[:],
                    op=Alu.subtract,
                )
                pt = stage.tile([P, HALF], f32, tag=f"pt{h}")
                nc.scalar.activation(out=pt[:], in_=logpt[:], func=Act.Exp)
                om = stage.tile([P, HALF], f32, tag=f"om{h}")  # 1 - pt
                nc.scalar.activation(
                    out=om[:], in_=pt[:], func=Act.Copy, scale=-1.0, bias=1.0
                )
                ln1m = stage.tile([P, HALF], f32, tag=f"ln1m{h}")
                nc.scalar.activation(out=ln1m[:], in_=om[:], func=Act.Ln)

                gamma = stage.tile([P, HALF], f32, tag=f"gamma{h}")
                nc.vector.memset(gamma[:], gammas[0])
                mtmp = stage.tile([P, HALF], f32, tag=f"mtmp{h}")
                for i in range(NUM_BINS - 1):
                    dg = gammas[i + 1] - gammas[i]
                    nc.vector.tensor_scalar(
                        out=mtmp[:], in0=pt[:], scalar1=edges[i], scalar2=dg,
                        op0=Alu.is_ge, op1=Alu.mult,
                    )
                    nc.vector.tensor_tensor(
                        out=gamma[:], in0=gamma[:], in1=mtmp[:], op=Alu.add
                    )

                prod = stage.tile([P, HALF], f32, tag=f"prod{h}")
                dve_absorb(ln1m[:, 0:1])
                nc.vector.tensor_tensor(
                    out=prod[:], in0=gamma[:], in1=ln1m[:], op=Alu.mult
                )
                focal = stage.tile([P, HALF], f32, tag=f"focal{h}")
                nc.scalar.activation(out=focal[:], in_=prod[:], func=Act.Exp)
                contrib = stage.tile([P, HALF], f32, tag=f"contrib{h}")
                dve_absorb(focal[:, 0:1])
                nc.vector.tensor_tensor(
                    out=contrib[:], in0=focal[:], in1=logpt[:], op=Alu.mult
                )
                nc.vector.tensor_reduce(
                    out=parts[:, h : h + 1], in_=contrib[:],
                    axis=mybir.AxisListType.X, op=Alu.add, negate=True,
                )

            # main streaming loop. Chunk 0 is issued as J separate block DMAs
            # so the first compute starts after ~0.25 MB.
            for k in range(CHUNKS):
                x_t = xpool.tile([P, J, C], bf16, tag="x")
                if k == 0:
                    for j in range(J):
                        nc.sync.dma_start(
                            out=x_t[:, j, :],
                            in_=x_re[0][:, j * C : (j + 1) * C],
                        )
                else:
                    nc.sync.dma_start(out=x_t[:], in_=x_re[k])
                last_acc_col = None
                for j in range(J):
                    col = k * J + j
                    if block_on_dve(col):
                        # Schraudolph exp2: the first DVE accessor of the
                        # chunk, so it naturally absorbs the DMA wait.
                        e_t = ipool.tile([P, C], i16, tag="sch")
                        nc.vector.tensor_scalar(
                            out=e_t[:], in0=x_t[:, j, :],
                            scalar1=SCH_C1, scalar2=SCH_C2,
                            op0=Alu.mult, op1=Alu.add,
                        )
                        nc.vector.tensor_reduce(
                            out=s_dve[:, col : col + 1],
                            in_=e_t[:].bitcast(bf16),
                            axis=mybir.AxisListType.X, op=Alu.add,
                        )
                    else:
                        e_t = epool.tile([P, C], bf16, tag="exp_out")
                        nc.scalar.activation(
                            out=e_t[:], in_=x_t[:, j, :], func=Act.Exp,
                            accum_out=s_acc[:, col : col + 1],
                        )
                        last_acc_col = col
                # cross-engine joiner: last DVE accessor of this x slot. Its
                # in0 column was produced by this chunk's last ACT exp, so
                # the joiner's completion implies every reader of the slot
                # is done and the slot-recycling DMA can carry a single DVE
                # wait.
                joiner = scratch.tile([P, 1], f32, tag="joiner")
                nc.vector.tensor_tensor(
                    out=joiner[:],
                    in0=s_acc[:, last_acc_col : last_acc_col + 1],
                    in1=x_t[:, 0, 0:2].bitcast(f32),
                    op=Alu.add,
                )
                if k == CHUNKS // 2 - 1:
                    # first epilogue half overlaps the remaining streaming
                    epilogue_half(0)

            epilogue_half(1)
            part = stage.tile([P, 1], f32, tag="part")
            nc.vector.tensor_tensor(
                out=part[:], in0=parts[:, 0:1], in1=parts[:, 1:2], op=Alu.add
            )
            nc.sync.dma_start(out=out_d[:, :], in_=part[:])

    if hw_fixups:
        apply_hw_fixups(nc, mybir)
    return nc


def apply_hw_fixups(nc, mybir):
    # Tile piggybacks an own-engine semaphore wait onto any instruction that
    # carries a cross-engine wait. Engines execute and complete their queue
    # in order (the DVE even drains its pipe between ops), so a wait on the
    # instruction's own engine semaphore is always redundant — strip it.
    own_prefix = {
        "EngineType.DVE": "DVE",
        "EngineType.Activation": "Activation",
        "EngineType.Pool": "Pool",
        "EngineType.PE": "PE",
        "EngineType.SP": "SP",
    }
    for blk in nc.m.functions[0].blocks:
        for ins in blk.instructions:
            si = getattr(ins, "sync_info", None)
            if si is None or type(ins).__name__ == "InstDMACopy":
                continue
            if len(si.on_wait) <= 1:
                continue
            pref = own_prefix.get(str(getattr(ins, "engine", "")), None)
            if pref is None:
                continue
            keep = [w for w in si.on_wait if not w.ant_name.startswith(pref + "_")]
            if len(keep) < len(si.on_wait):
                ins.sync_info = type(si)(on_wait=keep, on_update=list(si.on_update))

    # walrus' DMA instruction encoding holds a single sync wait. Tile puts
    # up to three on the steady-state streaming DMAs: the recycled slot's ACT
    # readers, its DVE readers, and its previous DMA writer (WAW). The DVE
    # wait alone is sufficient: the last DVE accessor is the per-chunk joiner
    # above, whose completion transitively implies the ACT readers and (via
    # the in-order DVE queue and the first DVE reader's DMA wait) the
    # previous writer.
    for blk in nc.m.functions[0].blocks:
        for ins in blk.instructions:
            si = getattr(ins, "sync_info", None)
            if si is None or type(ins).__name__ != "InstDMACopy":
                continue
            if len(si.on_wait) <= 1:
                continue
            keep = [w for w in si.on_wait if w.ant_name.startswith("DVE")]
            assert len(keep) == 1, (ins.name, [w.ant_name for w in si.on_wait])
            ins.sync_info = type(si)(on_wait=keep, on_update=list(si.on_update))

    # The kernel-tail drain aggregates one wait per semaphore in a single
    # instruction; split it into a chain of single-wait drains on the same
    # engine (sequential execution preserves the barrier semantics).
    for blk in nc.m.functions[0].blocks:
        il = blk.instructions
        i = 0
        while i < len(il):
            ins = il[i]
            si = getattr(ins, "sync_info", None)
            if (
                si is not None
                and type(ins).__name__ == "InstDrain"
                and len(si.on_wait) > 1
            ):
                SyncInfo = type(si)
                waits = list(si.on_wait)
                for k, w in enumerate(waits[:-1]):
                    d = mybir.InstDrain(
                        name=f"{ins.name}-w{k}", ins=[], outs=[],
                        bass_is_fusable=False,
                    )
                    d.engine = ins.engine
                    d.sync_info = SyncInfo(on_wait=[w], on_update=[])
                    il.insert(i, d)
                    i += 1
                ins.sync_info = SyncInfo(
                    on_wait=[waits[-1]], on_update=list(si.on_update)
                )
            i += 1


def make_in_maps(input, target):
    import ml_dtypes

    x = np.asarray(input, dtype=np.float32)
    t = np.asarray(target).astype(np.int64)
    xt = x[np.arange(N), t]                       # fp32 target logits
    xb = x.astype(ml_dtypes.bfloat16)
    in_maps = []
    for c in range(NCORES):
        xs = xb[c * RPC : (c + 1) * RPC]
        # interleave to DRAM order [k][p][j][c] so each partition's chunk
        # block is contiguous (row k*J*128 + j*128 + p -> [k][p][j])
        xi = np.ascontiguousarray(
            xs.reshape(CHUNKS, J, P, C).transpose(0, 2, 1, 3).reshape(RPC, C)
        )
        # xt laid out [P, COLS]: xt_tile[p, col] = xt[core_base + col*128 + p]
        xts = np.ascontiguousarray(
            xt[c * RPC : (c + 1) * RPC].reshape(COLS, P).T
        ).astype(np.float32)
        in_maps.append({"xin": xi, "xt": xts})
    return in_maps


def kernel(input, target, bin_edges, bin_gammas):
    global LAST_RESULT
    from concourse.bass_utils import run_bass_kernel_spmd

    nc = build_program(bin_edges, bin_gammas)
    in_maps = make_in_maps(input, target)
    trace = bool(os.environ.get("BASS_TRACE"))
    res = run_bass_kernel_spmd(nc, in_maps, list(range(NCORES)), trace=trace)
    LAST_RESULT = res
    total = np.float64(0.0)
    for r in res.results:
        total += r["out"].astype(np.float64).sum()
    return np.float32(total)
